# revision 15
# baseline (speedup 1.0000x reference)
"""Trainium2 Bass kernel for nn_GNOME_42588895707869 (GNN message passing + cdist).

Sharding: core k owns dst-nodes [1024k, 1024(k+1)) of BOTH graphs (local node
columns: 0-1023 = graph-1 slice, 1024-2047 = graph-2 slice). x rows of both
graphs are AllGathered (bf16) into x_rows[16384, H] with row id
r(n,g) = 2048*(n//1024) + 1024*g + n%1024, then DMA'd into SBUF.

Message passing avoids serialized dma_gathers: edges are bucketed host-side
into a uniform 16x16 grid of (dst-window wd, src-512-row-group j) cells, up to
128 edges per cell (class-1). x[src] is gathered on the TENSOR engine: per
chunk, 4 one-hot matmuls (one per 128-row src window) accumulate into PSUM on
top of an identity matmul that adds the edge-MLP output; scalar Relu emits the
bf16 message; a dst one-hot matmul scatter-adds into one of 16 PSUM
accumulators packed 4-per-bank. Cell overflow goes through one dma_gather per
layer. Node MLP in fp32r. cdist in bf16 with norms computed from the bf16 m
(consistent cancellation) and carried as bf16 hi+lo row pairs folded into the
contraction via a two-ones-row stationary block.
"""
import sys

sys.path.insert(0, "/opt/trn_rl_repo")

import numpy as np  # noqa: E402

N = 8192
H = 128
L = 6
CAT = 768
E = 131072
NQ = 2048          # local nodes per core (1024 per graph)
NS = 1024          # nodes per graph per core
WSZ = 128
C1 = 256           # class-1 chunks (16 wd x 16 j)
XIN = 64           # features(48) + RW(16)
ROWS_D = 1024      # cdist rows per core
EPS = 1e-12
MR = CAT + 2       # m rows (row 0 = nsq hi, row 1 = nsq lo)


def _rowid(src, g):
    return 2048 * (src // NS) + NS * g + (src % NS)


# ---------------------------------------------------------------- host prep
def _pack_all(ei1, ei2, ef1, ef2):
    """Bucket edges into per-core quarter-cells (wd, node-window) + overflow."""
    cores = []
    for k in range(8):
        cells = [[[] for _ in range(64)] for _ in range(16)]
        over = [[] for _ in range(16)]
        for g, (ei, ef) in enumerate(((ei1, ef1), (ei2, ef2))):
            src = np.asarray(ei[0]).astype(np.int64)
            dst = np.asarray(ei[1]).astype(np.int64)
            sel = (dst // NS) == k
            s_k, d_k = src[sel], dst[sel] - k * NS
            ef_k = np.asarray(ef, np.float32)[sel]
            r = _rowid(s_k, g)
            wd = d_k // WSZ + 8 * g
            wn = s_k // WSZ
            drel = d_k % WSZ
            for i in range(len(s_k)):
                cell = cells[wd[i]][wn[i]]
                if len(cell) < 32:
                    cell.append((r[i], drel[i], ef_k[i]))
                else:
                    over[wd[i]].append((r[i], drel[i], ef_k[i]))
        cores.append((cells, over))
    owc = max(max((len(o) + 127) // 128 for o in over) for cells, over in cores)
    owc = max(owc, 1)
    return cores, owc


def _core_layout(core, owc):
    import ml_dtypes
    cells, over = core
    C = C1 + 16 * owc
    oh = np.zeros((128, C1 * 128), dtype=ml_dtypes.bfloat16)
    ohd = np.zeros((128, C * 128), dtype=ml_dtypes.bfloat16)
    srco = np.zeros(16 * owc * 128, dtype=np.int16)
    ef_perm = np.zeros((C * 128, 9), dtype=np.float32)
    for wd in range(16):
        for a in range(16):
            c = wd * 16 + a
            base = c * 128
            for q in range(4):
                cell = cells[wd][4 * a + q]
                for e, (r, drel, ef) in enumerate(cell):
                    s = q * 32 + e
                    oh[int(r) % 128, base + s] = 1.0
                    ohd[s, base + int(drel)] = 1.0
                    ef_perm[base + s, :8] = ef
                    ef_perm[base + s, 8] = 1.0
        ov = over[wd]
        for oc in range(owc):
            c = C1 + wd * owc + oc
            base = c * 128
            seg = ov[oc * 128:(oc + 1) * 128]
            sbase = (wd * owc + oc) * 128
            for e, (r, drel, ef) in enumerate(seg):
                srco[sbase + e] = r
                ohd[e, base + int(drel)] = 1.0
                ef_perm[base + e, :8] = ef
                ef_perm[base + e, 8] = 1.0
    return oh, ohd, srco, np.ascontiguousarray(ef_perm.T)


def _idx_sb(idx):
    n = idx.shape[0]
    assert n % 16 == 0
    a = np.ascontiguousarray(idx.astype(np.int16).reshape(n // 16, 16).T)
    return np.tile(a, (8, 1)).copy()


# ---------------------------------------------------------------- program
_prog_cache = {}


def _build_program(OWC):
    import concourse.bass as bass  # noqa: F401
    import concourse.mybir as mybir
    from concourse import bacc
    from concourse.tile import TileContext
    from concourse.masks import make_identity

    f32 = mybir.dt.float32
    f32r = mybir.dt.float32r
    bf16 = mybir.dt.bfloat16
    i16 = mybir.dt.int16
    AF = mybir.ActivationFunctionType
    Alu = mybir.AluOpType

    C = C1 + 16 * OWC
    NOV = 16 * OWC                 # overflow chunks
    ES = 16                        # phase-A edge-MLP slab
    assert C % ES == 0

    nc = bacc.Bacc("TRN2", num_devices=8)

    xin = nc.declare_dram_parameter("xin", [XIN + 1, NQ], f32, isOutput=False)
    wpre = nc.declare_dram_parameter("wpre", [XIN + 1, H], f32, isOutput=False)
    wedge = nc.declare_dram_parameter("wedge", [9, H], f32, isOutput=False)
    efT = nc.declare_dram_parameter("efT", [9, C * 128], f32, isOutput=False)
    oh_d = nc.declare_dram_parameter("oh_d", [128, C1 * 128], bf16,
                                     isOutput=False)
    ohd_d = nc.declare_dram_parameter("ohd_d", [128, C * 128], bf16,
                                      isOutput=False)
    srcov = nc.declare_dram_parameter("srcov", [128, NOV * 8], i16,
                                      isOutput=False)
    gw1 = nc.declare_dram_parameter("gw1", [L, H, H], f32, isOutput=False)
    gw2 = nc.declare_dram_parameter("gw2", [L, H, H], f32, isOutput=False)
    gb1t = nc.declare_dram_parameter("gb1t", [H, L], f32, isOutput=False)
    gb2t = nc.declare_dram_parameter("gb2t", [H, L], f32, isOutput=False)
    wo1 = nc.declare_dram_parameter("wo1", [CAT, 2 * CAT], f32, isOutput=False)
    wo2 = nc.declare_dram_parameter("wo2", [2 * CAT, CAT], f32, isOutput=False)
    bo1t = nc.declare_dram_parameter("bo1t", [H, 12], f32, isOutput=False)
    bo2t = nc.declare_dram_parameter("bo2t", [H, 6], f32, isOutput=False)
    out = nc.declare_dram_parameter("out", [ROWS_D, N], f32, isOutput=True)


    x_rows = nc.dram_tensor("x_rows", [2 * N, H], bf16, addr_space="Shared")
    x_ag_in = nc.dram_tensor("x_ag_in", [NQ, H], bf16)
    e_hbm = nc.dram_tensor("e_hbm", [128, C * H], bf16)
    outs_hbm = nc.dram_tensor("outs_hbm", [L, H, NQ], f32)
    outs_hbm = outs_hbm.handle.bitcast(f32r).reshape([L, H, NQ]) if False else outs_hbm
    m_loc = nc.dram_tensor("m_loc", [MR, NS], bf16)
    mag_in = nc.dram_tensor("mag_in", [MR, NS], bf16)
    mT_all = nc.dram_tensor("mT_all", [8 * MR, NS], bf16, addr_space="Shared")

    allg = [[0, 1, 2, 3, 4, 5, 6, 7]]

    def G_of(wd, j):
        return 4 * (j // 2) + 2 * (wd // 8) + (j % 2)

    with TileContext(nc) as tc:
        cpool = tc.alloc_tile_pool(name="const", bufs=1)
        ident = cpool.tile([128, 128], f32)
        make_identity(nc, ident[:])
        identr = cpool.tile([128, 128], f32r)
        nc.vector.tensor_copy(identr[:], ident[:])
        identb = cpool.tile([128, 128], bf16)
        nc.vector.tensor_copy(identb[:], ident[:])
        gb1s = cpool.tile([H, L], f32)
        nc.sync.dma_start(out=gb1s[:], in_=gb1t[:])
        gb2s = cpool.tile([H, L], f32)
        nc.sync.dma_start(out=gb2s[:], in_=gb2t[:])
        w1r = cpool.tile([H, L, H], f32r)
        w2r = cpool.tile([H, L, H], f32r)
        xcur = cpool.tile([H, NQ], f32r)
        feat_t = cpool.tile([H, NQ], f32r)
        gpool = tc.alloc_tile_pool(name="grid", bufs=1)
        srcot = gpool.tile([128, NOV * 8], i16)
        nc.sync.dma_start(out=srcot[:], in_=srcov[:])
        aggT = gpool.tile([H, NQ], f32)
        X_sb = gpool.tile([128, 128, H], bf16)

        def x_exchange(pool, get_pst):
            for t in range(NQ // 128):
                pst = get_pst(t)
                nc.tensor.transpose(out=pst,
                                    in_=xcur[:, t * 128:(t + 1) * 128],
                                    identity=identr[:])
                xr = pool.tile([128, H], bf16, tag="xr")
                nc.scalar.activation(xr[:], pst, AF.Copy)
                nc.sync.dma_start(
                    out=x_ag_in[:].rearrange("(a p) m -> a p m", p=128)[t],
                    in_=xr[:])
            nc.gpsimd.collective_compute(
                "AllGather", Alu.bypass, ins=[x_ag_in[:]], outs=[x_rows[:]],
                replica_groups=allg)
            nc.sync.dma_start(
                out=X_sb[:],
                in_=x_rows[:].rearrange("(a p) m -> p a m", p=128))

        # ---------------- phase A ------------------------------------
        with tc.tile_pool(name="phA", bufs=2) as pa, \
             tc.tile_pool(name="psA", bufs=2, space="PSUM") as ppa:
            wtmp = pa.tile([H, L, H], f32, tag="wtmp")
            nc.sync.dma_start(out=wtmp[:], in_=gw1[:].rearrange("l k m -> k l m"))
            nc.vector.tensor_copy(w1r[:], wtmp[:])
            wtmp2 = pa.tile([H, L, H], f32, tag="wtmp")
            nc.sync.dma_start(out=wtmp2[:], in_=gw2[:].rearrange("l k m -> k l m"))
            nc.vector.tensor_copy(w2r[:], wtmp2[:])

            xinf = pa.tile([XIN + 1, NQ], f32, tag="xinf")
            nc.sync.dma_start(out=xinf[:], in_=xin[:])
            xinr = pa.tile([XIN + 1, NQ], f32r, tag="xinr")
            nc.vector.tensor_copy(xinr[:], xinf[:])
            wpref = pa.tile([XIN + 1, H], f32, tag="wpref")
            nc.sync.dma_start(out=wpref[:], in_=wpre[:])
            wprer = pa.tile([XIN + 1, H], f32r, tag="wprer")
            nc.vector.tensor_copy(wprer[:], wpref[:])
            for nt in range(NQ // 512):
                ps = ppa.tile([H, 512], f32, space="PSUM", tag="psx")
                nc.tensor.matmul(ps[:], lhsT=wprer[:],
                                 rhs=xinr[:, nt * 512:(nt + 1) * 512],
                                 start=True, stop=True)
                nc.vector.tensor_copy(xcur[:, nt * 512:(nt + 1) * 512], ps[:])
            nc.vector.tensor_copy(feat_t[:], xcur[:])

            def pst_a(t):
                pst_t = ppa.tile([128, 128], f32r, space="PSUM", tag="psT")
                return pst_t[:]
            x_exchange(pa, pst_a)

            # edge MLP -> e_hbm bf16 (streamed back per layer)
            weg = pa.tile([9, H], f32, tag="weg")
            nc.sync.dma_start(out=weg[:], in_=wedge[:])
            wegb = pa.tile([9, H], bf16, tag="wegb")
            nc.vector.tensor_copy(wegb[:], weg[:])
            for sl in range(C // ES):
                c0 = sl * ES
                eslab = pa.tile([9, ES * 128], f32, tag="eslab")
                nc.sync.dma_start(out=eslab[:],
                                  in_=efT[:, c0 * 128:(c0 + ES) * 128])
                eslabb = pa.tile([9, ES * 128], bf16, tag="eslabb")
                nc.vector.tensor_copy(eslabb[:], eslab[:])
                egs = pa.tile([128, ES, H], bf16, tag="egs")
                for cc in range(ES):
                    pse = ppa.tile([128, H], f32, space="PSUM", tag="pse")
                    nc.tensor.matmul(pse[:],
                                     lhsT=eslabb[:, cc * 128:(cc + 1) * 128],
                                     rhs=wegb[:], start=True, stop=True)
                    nc.scalar.activation(egs[:, cc, :], pse[:], AF.Copy)
                nc.sync.dma_start(
                    out=e_hbm[:, c0 * H:(c0 + ES) * H], in_=egs[:])

        # ---------------- phase B: 6 GNN layers ----------------------
        with tc.tile_pool(name="phB", bufs=2) as pb, \
             tc.tile_pool(name="mgB", bufs=3) as pmg, \
             tc.tile_pool(name="srB", bufs=4) as psr, \
             tc.tile_pool(name="ohB", bufs=2) as pob, \
             tc.tile_pool(name="agB", bufs=1, space="PSUM") as pag, \
             tc.tile_pool(name="psM", bufs=2, space="PSUM") as ppm:
            paggs = [pag.tile([128, 4, 128], f32, space="PSUM",
                              tag=f"agg{i}", name=f"agg{i}")
                     for i in range(4)]
            zc = cpool.tile([1, 128], bf16, name="zc")
            nc.vector.memset(zc[:], 0.0)
            zr = cpool.tile([1, 512], bf16, name="zr")
            nc.vector.memset(zr[:], 0.0)
            pworks = [pag.tile([128, 4, 128], f32, space="PSUM",
                               tag=f"pwork{i}", name=f"pwork{i}")
                      for i in range(2)]

            def agg_slot(wd):
                return paggs[wd // 4][:, wd % 4, :]

            def pst_b(t):
                return pworks[0][:, t % 4, :].bitcast(f32r)

            def W_of(wd, wn):
                return 16 * (wn // 8) + 8 * (wd // 8) + wn % 8

            for l in range(L):
                for i in range(4):
                    nc.tensor.matmul(paggs[i][:], lhsT=zc[:], rhs=zr[:],
                                     start=True, stop=False)
                for sl in range(16):         # 16 class-1 chunks per slab
                    ohsl = psr.tile([128, 16, 128], bf16, tag="ohsl")
                    nc.sync.dma_start(
                        out=ohsl[:],
                        in_=oh_d[:, sl * 2048:(sl + 1) * 2048].rearrange(
                            "p (a m) -> p a m", m=128))
                    ohdsl = psr.tile([128, 16, 128], bf16, tag="ohdsl")
                    nc.sync.dma_start(
                        out=ohdsl[:],
                        in_=ohd_d[:, sl * 2048:(sl + 1) * 2048].rearrange(
                            "p (a m) -> p a m", m=128))
                    esl = psr.tile([128, 16, H], bf16, tag="esl")
                    nc.sync.dma_start(
                        out=esl[:],
                        in_=e_hbm[:, sl * 16 * H:(sl + 1) * 16 * H].rearrange(
                            "p (a m) -> p a m", m=H))
                    for jj in range(0, 16, 4):
                        grp = jj // 4
                        pwork = pworks[grp % 2]
                        nc.tensor.matmul(pwork[:], lhsT=identb[:],
                                         rhs=esl[:, jj:jj + 4, :],
                                         start=True, stop=False)
                        for ci in range(4):
                            c = sl * 16 + jj + ci
                            wd, aa = c // 16, c % 16
                            for q in range(4):
                                Wx = W_of(wd, 4 * aa + q)
                                nc.tensor.matmul(
                                    pwork[32 * q:32 * q + 32, ci, :],
                                    lhsT=ohsl[:, jj + ci,
                                              32 * q:32 * q + 32],
                                    rhs=X_sb[:, Wx, :],
                                    start=False, stop=(q == 3),
                                    tile_position=(0, 32 * q))
                        msg4 = pmg.tile([128, 4, 128], bf16, tag="msg")
                        if grp % 2 == 0:
                            nc.scalar.activation(msg4[:], pwork[:], AF.Relu)
                        else:
                            nc.vector.tensor_relu(msg4[:], pwork[:])
                        for ci in range(4):
                            c = sl * 16 + jj + ci
                            wd = c // 16
                            nc.tensor.matmul(agg_slot(wd),
                                             lhsT=ohdsl[:, jj + ci, :],
                                             rhs=msg4[:, ci, :],
                                             start=False, stop=False)

                # --- overflow pass (one dma_gather for all 16 wds)
                og = pb.tile([128, NOV, H], bf16, tag="og", bufs=1)
                nc.gpsimd.dma_gather(og[:], x_rows[:], srcot[:],
                                     NOV * 128, NOV * 128, H, elem_step=H,
                                     single_packet=False)
                eov = pb.tile([128, NOV, H], bf16, tag="eov", bufs=1)
                nc.sync.dma_start(
                    out=eov[:],
                    in_=e_hbm[:, C1 * H:C * H].rearrange(
                        "p (a m) -> p a m", m=H))
                tmpo = pb.tile([128, NOV, H], bf16, tag="tmpo", bufs=1)
                nc.vector.tensor_tensor(out=tmpo[:], in0=og[:],
                                        in1=eov[:], op=Alu.add)
                msgo = pb.tile([128, NOV, H], bf16, tag="msgo", bufs=1)
                nc.scalar.activation(msgo[:], tmpo[:], AF.Relu)
                ohdo = pb.tile([128, NOV, WSZ], bf16, tag="ohdo", bufs=1)
                nc.sync.dma_start(
                    out=ohdo[:],
                    in_=ohd_d[:, C1 * 128:C * 128].rearrange(
                        "p (a m) -> p a m", m=128))
                for wd in range(16):
                    for oc in range(OWC):
                        nc.tensor.matmul(agg_slot(wd),
                                         lhsT=ohdo[:, wd * OWC + oc, :],
                                         rhs=msgo[:, wd * OWC + oc, :],
                                         start=False, stop=(oc == OWC - 1))

                # --- window epilogues: psum agg -> aggT columns
                for wd in range(16):
                    aggn = pb.tile([128, H], f32r, tag="aggn")
                    nc.scalar.activation(aggn[:], agg_slot(wd), AF.Copy)
                    psT = pworks[1][:, wd % 4, :].bitcast(f32r)
                    nc.tensor.transpose(out=psT, in_=aggn[:],
                                        identity=identr[:])
                    nc.vector.tensor_copy(aggT[:, wd * WSZ:(wd + 1) * WSZ],
                                          psT)

                # --- node MLP (own 2048 nodes)
                for nt in range(NQ // 512):
                    sl_ = slice(nt * 512, (nt + 1) * 512)
                    ht = pb.tile([H, 512], f32r, tag="ht")
                    nc.vector.tensor_tensor(out=ht[:], in0=xcur[:, sl_],
                                            in1=aggT[:, sl_], op=Alu.add)
                    ps1 = ppm.tile([H, 512], f32, space="PSUM", tag="psmlp")
                    nc.tensor.matmul(ps1[:], lhsT=w1r[:, l, :], rhs=ht[:],
                                     start=True, stop=True)
                    t1 = pb.tile([H, 512], f32r, tag="t1")
                    nc.scalar.activation(t1[:], ps1[:], AF.Relu,
                                         bias=gb1s[:, l:l + 1])
                    ps2 = ppm.tile([H, 512], f32, space="PSUM", tag="psmlp")
                    nc.tensor.matmul(ps2[:], lhsT=w2r[:, l, :], rhs=t1[:],
                                     start=True, stop=True)
                    if l in (1, 3):
                        s0 = pb.tile([H, 512], f32, space="SBUF", tag="s0")
                        nc.scalar.activation(s0[:], ps2[:], AF.Identity,
                                             bias=gb2s[:, l:l + 1])
                        nc.vector.tensor_tensor(out=feat_t[:, sl_], in0=s0[:],
                                                in1=feat_t[:, sl_], op=Alu.add)
                        nc.vector.tensor_relu(xcur[:, sl_], feat_t[:, sl_])
                    else:
                        nc.scalar.activation(xcur[:, sl_], ps2[:], AF.Relu,
                                             bias=gb2s[:, l:l + 1])
                nc.sync.dma_start(out=outs_hbm[l].bitcast(f32r), in_=xcur[:])
                if l < L - 1:
                    x_exchange(pb, pst_b)

        gpool.release()

        # ---------------- phase C: output MLP ------------------------
        with tc.tile_pool(name="phCw", bufs=1) as pcw, \
             tc.tile_pool(name="phC", bufs=2) as pc, \
             tc.tile_pool(name="phCh", bufs=1) as pch, \
             tc.tile_pool(name="psC", bufs=4, space="PSUM") as ppc:
            wo1r = pcw.tile([128, 6, 2 * CAT], f32r, tag="wo1r")
            wo2r = pcw.tile([128, 12, CAT], f32r, tag="wo2r")
            for kc in range(6):
                wt = pc.tile([128, 2 * CAT], f32, tag="wldtmp")
                nc.sync.dma_start(
                    out=wt[:],
                    in_=wo1[:].rearrange("(a p) m -> a p m", p=128)[kc])
                nc.vector.tensor_copy(wo1r[:, kc, :], wt[:])
            for kc in range(12):
                wt = pc.tile([128, CAT], f32, tag="wldtmp")
                nc.sync.dma_start(
                    out=wt[:],
                    in_=wo2[:].rearrange("(a p) m -> a p m", p=128)[kc])
                nc.vector.tensor_copy(wo2r[:, kc, :], wt[:])
            bo1s = pcw.tile([H, 12], f32, tag="bo1s")
            nc.sync.dma_start(out=bo1s[:], in_=bo1t[:])
            bo2s = pcw.tile([H, 6], f32, tag="bo2s")
            nc.sync.dma_start(out=bo2s[:], in_=bo2t[:])
            ones_r = pcw.tile([128, 1], f32r, tag="ones_r")
            onesf = pcw.tile([128, 1], f32, tag="onesf")
            nc.vector.memset(onesf[:], 1.0)
            nc.vector.tensor_copy(ones_r[:], onesf[:])

            nsq_sb = pcw.tile([1, NQ], f32, tag="nsq_sb")
            for nt in (2, 3, 0, 1):
                sl_ = slice(nt * 512, (nt + 1) * 512)
                ne_t = []
                for kc in range(6):
                    nt_t = pc.tile([H, 512], f32r, tag=f"ne{kc}")
                    nc.sync.dma_start(out=nt_t[:], in_=outs_hbm[kc][:, sl_].bitcast(f32r))
                    ne_t.append(nt_t)
                h1 = pch.tile([128, 12, 512], f32r, tag="h1")
                for mt in range(12):
                    ps = ppc.tile([128, 512], f32, space="PSUM", tag="psc")
                    for kc in range(6):
                        nc.tensor.matmul(
                            ps[:], lhsT=wo1r[:, kc, mt * 128:(mt + 1) * 128],
                            rhs=ne_t[kc][:], start=(kc == 0), stop=(kc == 5))
                    nc.scalar.activation(h1[:, mt, :], ps[:], AF.Relu,
                                         bias=bo1s[:, mt:mt + 1])
                sqsum = ppc.tile([1, 512], f32, space="PSUM", tag="sqsum")
                # local cols 0-1023 -> m_loc, 1024-2047 -> mag_in
                mdst = m_loc if nt < 2 else mag_in
                coff = nt * 512 if nt < 2 else (nt - 2) * 512
                for m2 in range(6):
                    ps = ppc.tile([128, 512], f32, space="PSUM", tag="psc")
                    for kc in range(12):
                        nc.tensor.matmul(
                            ps[:], lhsT=wo2r[:, kc, m2 * 128:(m2 + 1) * 128],
                            rhs=h1[:, kc, :], start=(kc == 0), stop=(kc == 11))
                    mtile = pc.tile([128, 512], f32, tag="mtile")
                    nc.scalar.activation(mtile[:], ps[:], AF.Identity,
                                         bias=bo2s[:, m2:m2 + 1])
                    mtileb = pc.tile([128, 512], bf16, tag="mtileb")
                    nc.vector.tensor_copy(mtileb[:], mtile[:])
                    nc.sync.dma_start(
                        out=mdst[2 + m2 * 128:2 + (m2 + 1) * 128,
                                 coff:coff + 512],
                        in_=mtileb[:])
                    sq = pc.tile([128, 512], f32r, tag="sq")
                    nc.vector.tensor_tensor(out=sq[:], in0=mtileb[:],
                                            in1=mtileb[:], op=Alu.mult)
                    nc.tensor.matmul(sqsum[:], lhsT=ones_r[:], rhs=sq[:],
                                     start=(m2 == 0), stop=(m2 == 5))
                nc.vector.tensor_copy(nsq_sb[:, sl_], sqsum[:])
            nsqb = pcw.tile([1, NQ], bf16, tag="nsqb")
            nlo = pcw.tile([1, NQ], f32, tag="nlo")
            nlob = pcw.tile([1, NQ], bf16, tag="nlob")
            nc.vector.tensor_copy(nsqb[:, NS:NQ], nsq_sb[:, NS:NQ])
            nc.vector.tensor_tensor(out=nlo[:, NS:NQ], in0=nsq_sb[:, NS:NQ],
                                    in1=nsqb[:, NS:NQ], op=Alu.subtract)
            nc.vector.tensor_copy(nlob[:, NS:NQ], nlo[:, NS:NQ])
            nc.sync.dma_start(out=mag_in[0:1, :], in_=nsqb[:, NS:NQ])
            nc.sync.dma_start(out=mag_in[1:2, :], in_=nlob[:, NS:NQ])
            nc.gpsimd.collective_compute(
                "AllGather", Alu.bypass, ins=[mag_in[:]], outs=[mT_all[:]],
                replica_groups=allg)
            nc.vector.tensor_copy(nsqb[:, 0:NS], nsq_sb[:, 0:NS])
            nc.vector.tensor_tensor(out=nlo[:, 0:NS], in0=nsq_sb[:, 0:NS],
                                    in1=nsqb[:, 0:NS], op=Alu.subtract)
            nc.vector.tensor_copy(nlob[:, 0:NS], nlo[:, 0:NS])
            nc.sync.dma_start(out=m_loc[0:1, :], in_=nsqb[:, 0:NS])
            nc.sync.dma_start(out=m_loc[1:2, :], in_=nlob[:, 0:NS])

        # ---------------- phase D: cdist -----------------------------
        with tc.tile_pool(name="phD1", bufs=1) as pd1, \
             tc.tile_pool(name="phD", bufs=2) as pd, \
             tc.tile_pool(name="ohD", bufs=3) as pdd, \
             tc.tile_pool(name="psD", bufs=4, space="PSUM") as ppd:
            onesrow = pd1.tile([128, 128], bf16, tag="onesrow")
            nc.vector.memset(onesrow[:], 0.0)
            nc.vector.memset(onesrow[0:2, :], 1.0)
            m1pre = pd1.tile([128, 6, NS], bf16, tag="m1pre")
            nc.sync.dma_start(
                out=m1pre[:],
                in_=m_loc[2:2 + CAT, :].rearrange("(a p) m -> p a m", p=128))
            m1r = pd1.tile([128, 6, NS], bf16, tag="m1r")
            nc.vector.tensor_scalar_mul(m1r[:], m1pre[:], -2.0)
            n1hi = pd1.tile([128, 8], bf16, tag="n1hi")
            nc.sync.dma_start(
                out=n1hi[:],
                in_=m_loc[0:1, :].rearrange("o (b p) -> (o p) b", p=128))
            n1lo = pd1.tile([128, 8], bf16, tag="n1lo")
            nc.sync.dma_start(
                out=n1lo[:],
                in_=m_loc[1:2, :].rearrange("o (b p) -> (o p) b", p=128))
            n1f = pd1.tile([128, 8], f32, tag="n1f")
            nc.vector.tensor_tensor(out=n1f[:], in0=n1hi[:], in1=n1lo[:],
                                    op=Alu.add)
            epsf = pd1.tile([128, 8], f32, tag="epsf")
            nc.vector.tensor_scalar(out=epsf[:], in0=n1f[:], scalar1=-1.0,
                                    scalar2=EPS, op0=Alu.mult, op1=Alu.add)

            for rb in range(8):
                base = rb * MR
                st_r = pd.tile([128, 6, NS], bf16, tag="st_r")
                nc.sync.dma_start(
                    out=st_r[:],
                    in_=mT_all[base + 2:base + 2 + CAT, :].rearrange(
                        "(a p) m -> p a m", p=128))
                st6 = pd.tile([128, NS], bf16, tag="st6")
                nc.sync.dma_start(out=st6[:],
                                  in_=mT_all[base:base + 128, :])
                for b in range(8):
                    for hh in range(2):
                        soff = hh * 512
                        psd = ppd.tile([128, 512], f32, space="PSUM",
                                       tag="psd")
                        for kc in range(6):
                            nc.tensor.matmul(
                                psd[:],
                                lhsT=m1r[:, kc, b * 128:(b + 1) * 128],
                                rhs=st_r[:, kc, soff:soff + 512],
                                start=(kc == 0), stop=False)
                        nc.tensor.matmul(psd[:], lhsT=onesrow[:],
                                         rhs=st6[:, soff:soff + 512],
                                         start=False, stop=True)
                        s1 = pdd.tile([128, 512], f32, tag="s1")
                        nc.vector.tensor_scalar(out=s1[:], in0=psd[:],
                                                scalar1=epsf[:, b:b + 1],
                                                scalar2=0.0,
                                                op0=Alu.max, op1=Alu.add)
                        dt_ = pdd.tile([128, 512], f32, tag="dt_")
                        nc.scalar.activation(dt_[:], s1[:], AF.Sqrt,
                                             bias=n1f[:, b:b + 1])
                        nc.sync.dma_start(
                            out=out[b * 128:(b + 1) * 128,
                                    rb * 1024 + soff:rb * 1024 + soff + 512],
                            in_=dt_[:])
        cpool.release()

    nc.compile()
    return nc


# ---------------------------------------------------------------- entry
def kernel(**inputs):
    from concourse.bass_utils import run_bass_kernel_spmd

    cores, owc = _pack_all(inputs["edge_index_1"], inputs["edge_index_2"],
                           inputs["e_features1"], inputs["e_features2"])

    feats = [np.asarray(inputs["features_1"], dtype=np.float32),
             np.asarray(inputs["features_2"], dtype=np.float32)]
    rws = [np.asarray(inputs["RW_1"], dtype=np.float32),
           np.asarray(inputs["RW_2"], dtype=np.float32)]

    wpre_aug = np.vstack([np.asarray(inputs["W_pre"], dtype=np.float32),
                          np.asarray(inputs["b_pre"], dtype=np.float32)[None]])
    wedge_aug = np.vstack([np.asarray(inputs["W_edge"], dtype=np.float32),
                           np.asarray(inputs["b_edge"], dtype=np.float32)[None]])
    gw1 = np.asarray(inputs["gnn_w1"], dtype=np.float32)
    gw2 = np.asarray(inputs["gnn_w2"], dtype=np.float32)
    gb1t = np.ascontiguousarray(np.asarray(inputs["gnn_b1"], np.float32).T)
    gb2t = np.ascontiguousarray(np.asarray(inputs["gnn_b2"], np.float32).T)
    wo1 = np.asarray(inputs["W_out1"], dtype=np.float32)
    wo2 = np.asarray(inputs["W_out2"], dtype=np.float32)
    bo1t = np.ascontiguousarray(
        np.asarray(inputs["b_out1"], np.float32).reshape(12, 128).T)
    bo2t = np.ascontiguousarray(
        np.asarray(inputs["b_out2"], np.float32).reshape(6, 128).T)

    in_maps = []
    for k in range(8):
        oh, ohd, srco, efT = _core_layout(cores[k], owc)
        xg1 = np.concatenate(
            [feats[0][k * NS:(k + 1) * NS], rws[0][k * NS:(k + 1) * NS],
             np.ones((NS, 1), np.float32)], axis=1)
        xg2 = np.concatenate(
            [feats[1][k * NS:(k + 1) * NS], rws[1][k * NS:(k + 1) * NS],
             np.ones((NS, 1), np.float32)], axis=1)
        xin = np.concatenate([xg1, xg2], axis=0).T.copy()
        in_maps.append({
            "xin": np.ascontiguousarray(xin),
            "wpre": wpre_aug, "wedge": wedge_aug,
            "efT": efT,
            "oh_d": oh,
            "ohd_d": ohd,
            "srcov": _idx_sb(srco),
            "gw1": gw1, "gw2": gw2, "gb1t": gb1t, "gb2t": gb2t,
            "wo1": wo1, "wo2": wo2, "bo1t": bo1t, "bo2t": bo2t,
        })

    if owc not in _prog_cache:
        _prog_cache[owc] = _build_program(owc)
    nc = _prog_cache[owc]
    res = run_bass_kernel_spmd(nc, in_maps, list(range(8)), **_run_kwargs)
    global _last_result
    _last_result = res
    return np.vstack([np.asarray(res.results[k]["out"]) for k in range(8)])


_run_kwargs = {}
_last_result = None
